# revision 20
# baseline (speedup 1.0000x reference)
"""Trainium2 Bass kernel for nn_CLLayer (SimCLR-style contrastive loss).

Math (reference, tau=0.5):
    h1 = elu(z1 @ W1.T + b1) @ W2.T + b2 ; h2 likewise
    n1, n2 = row-normalized h1, h2
    l1_i = log(sum_j exp(2*n1_i.n1_j) + sum_j exp(2*n1_i.n2_j) - e^2) - 2*n1_i.n2_i
    l2_i = log(sum_j exp(2*n2_i.n2_j) + colsum_i(exp(2*S12)) - e^2) - 2*n1_i.n2_i
    out = 0.5*(l1+l2)

Sharding: row-parallel over N=8192 (1024 rows/core, 8 cores).

All matmuls run in fp8e4 DoubleRow mode (2x PE rate): host quantizes
W (x16) and z to fp8; device quantizes normalized embeddings (x16) to
fp8 and AllGathers them (1MB/core per tensor).

Work split per core (identical on every core; rank enters only through
partition_id-driven dynamic DMA offsets):
  - S12 strip: all 8 column blocks (no symmetry).
  - S11: diag block + rotated offsets o=1..3 (colsums shared to row
    owners via ReduceScatter) + offset 4 computed rowsum-only on both
    ends of the pair.  S22 mirrored (offsets 5..7 shared, 4 unshared).
  - 18 block-products/core instead of 24; diag blocks need no gathered
    data so they run while the AllGathers are in flight.
Three colsum-share vectors ride one fused [3,N] ReduceScatter.

Host-side prep: K-major transposes, fp8 casts (weights x16, descaled on
device via activation scale=1/16), and the ELU "-1" folded into an
adjusted fc2 bias computed from the *quantized* W2 so the fold is exact.
"""

import math
import os
from functools import lru_cache

import ml_dtypes
import numpy as np

import concourse.bacc as bacc
import concourse.bass as bass
import concourse.bass_isa as bass_isa
import concourse.mybir as mybir
import concourse.tile as tile
from concourse.bass_utils import run_bass_kernel_spmd

N, D = 8192, 1024
NCORES = 8
BLK = N // NCORES  # 1024
P = 128
KO = D // P  # 8 k-tiles
NT = BLK // P  # 8 i-tiles per core
E2 = float(np.exp(2.0))  # exp(1/tau), tau=0.5
BF = mybir.dt.bfloat16
F32 = mybir.dt.float32
F8 = mybir.dt.float8e4
AF = mybir.ActivationFunctionType
ALU = mybir.AluOpType
PM2 = mybir.MatmulPerfMode.DoubleRow
WS = 16.0  # host fp8 weight scale (descaled via activation scale)
IWS = 1.0 / WS
QS = 16.0  # fp8 quant scale for normalized embeddings (entries ~N(0, 1/4))
IQS2 = 1.0 / (QS * QS)  # sim psum holds 256*S


def _build():
    nc = bacc.Bacc("TRN2", target_bir_lowering=False, debug=False, num_devices=NCORES)

    z1t = nc.dram_tensor("z1t", [D, BLK], F8, kind="ExternalInput")
    z2t = nc.dram_tensor("z2t", [D, BLK], F8, kind="ExternalInput")
    w1t = nc.dram_tensor("w1t", [D, D], F8, kind="ExternalInput")
    w2t = nc.dram_tensor("w2t", [D, D], F8, kind="ExternalInput")
    b1 = nc.dram_tensor("b1", [D], F32, kind="ExternalInput")
    b2p = nc.dram_tensor("b2p", [D], F32, kind="ExternalInput")
    out = nc.dram_tensor("out", [BLK], F32, kind="ExternalOutput")

    kp = lambda ap: ap.rearrange("(ko ki) x -> ki ko x", ki=P)  # K-major -> [128, KO, x]
    pt = lambda ap: ap.rearrange("(t p) -> p t", p=P)  # [1024] -> [128, 8]

    with tile.TileContext(nc) as tc:
        with (
            tc.tile_pool(name="consts", bufs=1) as consts,
            tc.tile_pool(name="mats", bufs=1) as mats,
            tc.tile_pool(name="strip", bufs=1) as strip,
            tc.tile_pool(name="scratch", bufs=2) as scratch,
            tc.tile_pool(name="rhs", bufs=3) as rhsp,
            tc.tile_pool(name="expp", bufs=2) as expp,
            tc.tile_pool(name="small", bufs=1) as small,
            tc.tile_pool(name="psA", bufs=3, space="PSUM") as psA,
            tc.tile_pool(name="psB", bufs=2, space="PSUM") as psB,
            tc.tile_pool(name="dram", bufs=1, space="DRAM") as dram,
        ):
            pid_s = nc.sync.partition_id()
            pid_g = nc.gpsimd.partition_id()

            # ---------------- constants (z1/w1 first: they gate matmul #1) ----------
            w1_sb = consts.tile([P, KO, D], F8)
            w2_sb = consts.tile([P, KO, D], F8)
            z_sb = mats.tile([P, KO, BLK], F8, tag="zt")
            nc.sync.dma_start(z_sb[:], kp(z1t[:]))
            for ot in range(KO):
                nc.sync.dma_start(
                    w1_sb[:, :, bass.ts(ot, P)], kp(w1t[:])[:, :, bass.ts(ot, P)]
                )
            b1_sb = consts.tile([P, KO], F32)
            b2_sb = consts.tile([P, KO], F32)
            nc.sync.dma_start(b1_sb[:], pt(b1[:]))
            nc.sync.dma_start(b2_sb[:], pt(b2p[:]))
            ones_bf = consts.tile([P, 1], BF)
            nc.vector.memset(ones_bf[:], 1.0)
            for ot in range(KO):
                nc.sync.dma_start(
                    w2_sb[:, :, bass.ts(ot, P)], kp(w2t[:])[:, :, bass.ts(ot, P)]
                )
            z2a = rhsp.tile([P, KO, 512], F8, tag="rhs", name="z2a")
            z2b = rhsp.tile([P, KO, 512], F8, tag="rhs", name="z2b")
            nc.sync.dma_start(z2a[:], kp(z2t[:, 0:512]))
            nc.sync.dma_start(z2b[:], kp(z2t[:, 512:1024]))

            h1 = [mats.tile([P, BLK], BF, tag=f"h1_{j}", name=f"h1_{j}") for j in range(KO)]
            h2 = [mats.tile([P, BLK], BF, tag=f"h2_{j}", name=f"h2_{j}") for j in range(KO)]
            elu1 = mats.tile([P, KO, BLK], F8, tag="elu")
            # kt-pair granularity: diag sims can start before the full tensor
            # is quantized (per-pair deps instead of whole-tile)
            n1q = [mats.tile([P, 2, BLK], F8, tag=f"n1q{p}", name=f"n1q{p}") for p in range(4)]
            n2q = [mats.tile([P, 2, BLK], F8, tag=f"n2q{p}", name=f"n2q{p}") for p in range(4)]

            ag1_in = dram.tile([D, BLK], F8)
            ag2_in = dram.tile([D, BLK], F8)
            ag1_out = dram.tile([NCORES, D, BLK], F8, addr_space="Shared")
            ag2_out = dram.tile([NCORES, D, BLK], F8, addr_space="Shared")
            # rs rows: 0 = S12 colsums, 1 = S11 colsum shares, 2 = S22 colsum shares
            # (leading dim = destination core: RS scatters contiguous chunks)
            rs_in = dram.tile([NCORES, 3, BLK], F32)
            rs_out = dram.tile([3, BLK], F32)
            p_dram = dram.tile([BLK], F32)

            zrow = small.tile([1, BLK], F32, tag="zrow")
            nc.vector.memset(zrow[:], 0.0)

            # ------- projection + normalize (h per-ot bf16, fp8 QS*n into nq_sb) -----
            def project(z_at, elu_sb, h_ot, nq_sb, rn_slot):
                # layer 1: a1T[o, i] = W1T.T @ zT (K=d);
                # elu+1 = relu(a+b1) + min(exp(a+b1), 1); psum holds 16*a
                for ot in range(KO):
                    ps = psA.tile([P, 1024], F32, tag="ps_big")
                    for ch in range(2):
                        sl = bass.ts(ch, 512)
                        for kt in range(0, KO, 2):
                            nc.tensor.matmul(
                                ps[:, sl],
                                w1_sb[:, kt : kt + 2, bass.ts(ot, P)],
                                z_at(kt, ch),
                                start=(kt == 0),
                                stop=(kt == KO - 2),
                                perf_mode=PM2,
                            )
                    bcol = b1_sb[:, ot : ot + 1]
                    e_t = scratch.tile([P, 1024], F32, tag="e_t")
                    r_t = scratch.tile([P, 1024], F32, tag="r_t")
                    nc.scalar.activation(e_t[:], ps[:], AF.Exp, bias=bcol, scale=IWS)
                    nc.scalar.activation(r_t[:], ps[:], AF.Relu, bias=bcol, scale=IWS)
                    nc.vector.tensor_scalar(e_t[:], e_t[:], 1.0, None, ALU.min)
                    nc.vector.tensor_tensor(elu_sb[:, ot, :], e_t[:], r_t[:], ALU.add)
                # layer 2 -> h_ot (bf16), squares fired per-ot so sumsq pipelines
                sq_ot = []
                for ot in range(KO):
                    ps = psA.tile([P, 1024], F32, tag="ps_big")
                    for ch in range(2):
                        sl = bass.ts(ch, 512)
                        for kt in range(0, KO, 2):
                            nc.tensor.matmul(
                                ps[:, sl],
                                w2_sb[:, kt : kt + 2, bass.ts(ot, P)],
                                elu_sb[:, kt : kt + 2, bass.ds(ch * 512, 512)],
                                start=(kt == 0),
                                stop=(kt == KO - 2),
                                perf_mode=PM2,
                            )
                    nc.vector.tensor_scalar(
                        h_ot[ot][:], ps[:], IWS, b2_sb[:, ot : ot + 1], ALU.mult, ALU.add
                    )
                    sq = scratch.tile([P, BLK], BF, tag=f"sq{ot}", bufs=1)
                    nc.scalar.activation(sq[:], h_ot[ot][:], AF.Square)
                    sq_ot.append(sq)
                # sumsq over d (partitions) via ones-matmul
                ssps = [
                    psB.tile([1, 512], F32, name=f"ssps{rn_slot}_{c}", tag="ps_small")
                    for c in range(2)
                ]
                for kt in range(KO):
                    for ch in range(2):
                        nc.tensor.matmul(
                            ssps[ch][:],
                            ones_bf[:],
                            sq_ot[kt][:, bass.ts(ch, 512)],
                            start=(kt == 0),
                            stop=(kt == KO - 1),
                        )
                # rn = QS/||h|| per column: 1/sqrt(ssq/QS^2)
                rn_bf = small.tile([1, BLK], BF, tag="rn_bf")
                nrm = small.tile([1, BLK], F32, tag="nrm")
                for ch in range(2):
                    nc.scalar.activation(
                        nrm[:, bass.ts(ch, 512)], ssps[ch][:], AF.Sqrt, scale=IQS2
                    )
                with nc.allow_low_precision(reason="rn lands in bf16 regardless"):
                    nc.vector.reciprocal(rn_bf[:], nrm[:])
                rn_bc = scratch.tile([P, BLK], BF, tag="rnbc")
                nc.gpsimd.partition_broadcast(rn_bc[:], rn_bf[:])
                for kt in range(KO):
                    nc.vector.tensor_tensor(
                        nq_sb[kt // 2][:, kt % 2, :], h_ot[kt][:], rn_bc[:], ALU.mult
                    )

            project(lambda kt, ch: z_sb[:, kt : kt + 2, bass.ds(ch * 512, 512)], elu1, h1, n1q, 0)
            for p in range(4):
                nc.sync.dma_start(kp(ag1_in[:])[:, 2 * p : 2 * p + 2, :], n1q[p][:])
            rg = [list(range(NCORES))]
            nc.gpsimd.collective_compute(
                "AllGather", ALU.bypass, replica_groups=rg,
                ins=[ag1_in[:].opt()], outs=[ag1_out[:].opt()],
            )
            # elu2 reuses the z1 slot (z1 dead after its layer 1)
            elu2 = mats.tile([P, KO, BLK], F8, tag="zt", name="elu2")
            project(lambda kt, ch: (z2a if ch == 0 else z2b)[:, kt : kt + 2, :], elu2, h2, n2q, 1)
            for p in range(4):
                nc.sync.dma_start(kp(ag2_in[:])[:, 2 * p : 2 * p + 2, :], n2q[p][:])
            nc.gpsimd.collective_compute(
                "AllGather", ALU.bypass, replica_groups=rg,
                ins=[ag2_in[:].opt()], outs=[ag2_out[:].opt()],
            )
            # zero the rs slots no share will write (rank-rotated complements);
            # queued after the AG triggers: dyn DMAs cost ~4us each on gpsimd
            for o in (0, 4, 5, 6, 7):
                nc.gpsimd.dma_start(rs_in[bass.ds((pid_g + o) % 8, 1), 1, :], zrow[:])
            for o in (0, 1, 2, 3, 4):
                nc.gpsimd.dma_start(rs_in[bass.ds((pid_g + o) % 8, 1), 2, :], zrow[:])

            # rowsum partials: slot 0 = diag block, slots 1.. = rotated offsets
            r11p = strip.tile([P, NT, 5], F32)
            r12p = strip.tile([P, NT, 8], F32)
            r22p = strip.tile([P, NT, 5], F32)

            def sim_iter(lhs, tt, rt_at, accum, cs_t=None, first=False):
                ps = psA.tile([P, 1024], F32, tag="ps_big", name="ps_sim")
                for ch in range(2):
                    sl = bass.ts(ch, 512)
                    for p in range(4):
                        nc.tensor.matmul(
                            ps[:, sl],
                            lhs[p][:, :, bass.ts(tt, P)],
                            rt_at(p, ch),
                            start=(p == 0),
                            stop=(p == 3),
                            perf_mode=PM2,
                        )
                ex = expp.tile([P, 1024], BF, tag="ex")
                nc.scalar.activation(ex[:], ps[:], AF.Exp, scale=2.0 * IQS2, accum_out=accum)
                if cs_t is not None:
                    if first:
                        nc.vector.tensor_copy(cs_t[:], ex[:])
                    else:
                        nc.vector.tensor_tensor(cs_t[:], cs_t[:], ex[:], ALU.add)

            def colsum_flush(cs_t, row, o, nm):
                cs_r = scratch.tile([P, BLK], F32, tag="csr", name=f"csr_{nm}")
                nc.gpsimd.partition_all_reduce(cs_r[:], cs_t[:], P, bass_isa.ReduceOp.add)
                nc.gpsimd.dma_start(rs_in[bass.ds((pid_g + o) % 8, 1), row, :], cs_r[0:1, :])

            def cs_tile(nm):
                return scratch.tile([P, BLK], BF, tag="cs", bufs=3, name=f"cs_{nm}")

            # ---- diag blocks (local rhs; run while AllGathers are in flight) ----
            # S11 diag first: it only needs n1q, so it overlaps proj2's tail.
            loc1 = lambda p, ch: n1q[p][:, :, bass.ds(ch * 512, 512)]
            loc2 = lambda p, ch: n2q[p][:, :, bass.ds(ch * 512, 512)]
            for tt in range(NT):
                sim_iter(n1q, tt, loc1, r11p[:, tt, 0:1])
            cs_d = cs_tile("d12")
            for tt in range(NT):
                sim_iter(n1q, tt, loc2, r12p[:, tt, 0:1], cs_d, first=(tt == 0))
            colsum_flush(cs_d, 0, 0, "d12")

            # ---- p_i = n1_i . n2_i (local diag of S12; psum holds 256*p) ----
            pps = [psB.tile([1, 512], F32, name=f"pps{_c}", tag="ps_small") for _c in range(2)]
            for kt in range(KO):
                q = scratch.tile([P, BLK], BF, tag="pq")
                nc.vector.tensor_tensor(
                    q[:], n1q[kt // 2][:, kt % 2, :], n2q[kt // 2][:, kt % 2, :], ALU.mult
                )
                for ch in range(2):
                    nc.tensor.matmul(
                        pps[ch][:], ones_bf[:], q[:, bass.ts(ch, 512)],
                        start=(kt == 0), stop=(kt == KO - 1),
                    )
            for ch in range(2):
                p_c = small.tile([1, 512], F32, tag="ssq_c", name=f"p_c{ch}")
                nc.vector.tensor_copy(p_c[:], pps[ch][:])
                nc.gpsimd.dma_start(p_dram[ch * 512 : (ch + 1) * 512], p_c[:])

            def rhs_pair_dyn(ag, o, nm):
                idx = (pid_s + o) % 8
                a = rhsp.tile([P, KO, 512], F8, tag="rhs", name=f"ra_{nm}")
                b = rhsp.tile([P, KO, 512], F8, tag="rhs", name=f"rb_{nm}")
                blk = ag[bass.ds(idx, 1)].rearrange("one (ko ki) x -> ki (one ko) x", ki=P)
                nc.sync.dma_start(a[:], blk[:, :, 0:512])
                nc.sync.dma_start(b[:], blk[:, :, 512:1024])
                return lambda p, ch: (a if ch == 0 else b)[:, 2 * p : 2 * p + 2, :]

            # ---- pass A: S11 rotated offsets 1..4 (colsums shared for 1..3) ----
            for o in (1, 2, 3, 4):
                rt = rhs_pair_dyn(ag1_out, o, f"A{o}")
                cs_t = cs_tile(f"a{o}") if o < 4 else None
                for tt in range(NT):
                    sim_iter(n1q, tt, rt, r11p[:, tt, o : o + 1], cs_t, first=(tt == 0))
                if o < 4:
                    colsum_flush(cs_t, 1, o, f"a{o}")

            # ---- pass B: S12 offsets 1..7 (+ S22 on 4..7, sharing the rhs load) ----
            for o in range(1, 8):
                rt = rhs_pair_dyn(ag2_out, o, f"B{o}")
                cs12 = cs_tile(f"b{o}")
                for tt in range(NT):
                    sim_iter(n1q, tt, rt, r12p[:, tt, o : o + 1], cs12, first=(tt == 0))
                colsum_flush(cs12, 0, o, f"b{o}")
                if o >= 5:
                    slot = o - 3  # r22p slots 2..4
                    cs22 = cs_tile(f"c{o}")
                    for tt in range(NT):
                        sim_iter(
                            n2q, tt, rt, r22p[:, tt, slot : slot + 1],
                            cs22, first=(tt == 0),
                        )
                    colsum_flush(cs22, 2, o, f"c{o}")

            # RS-independent denominator partials: overlap the collective
            r11 = small.tile([P, NT], F32, tag="r11")
            r12 = small.tile([P, NT], F32, tag="r12")
            d1p = small.tile([P, NT], F32, tag="d1")
            nc.vector.reduce_sum(r11[:], r11p[:], axis=mybir.AxisListType.X)
            nc.vector.reduce_sum(r12[:], r12p[:], axis=mybir.AxisListType.X)
            nc.vector.tensor_tensor(d1p[:], r11[:], r12[:], ALU.add)
            nc.vector.tensor_scalar(d1p[:], d1p[:], -E2, None, ALU.add)

            nc.gpsimd.collective_compute(
                "ReduceScatter", ALU.add, replica_groups=rg,
                ins=[rs_in[:].opt()], outs=[rs_out[:].opt()],
            )

            # S22 diag + offset-4 blocks (rowsum-only, no colsum share):
            # run during the RS / cross-core drain
            for tt in range(NT):
                sim_iter(n2q, tt, loc2, r22p[:, tt, 0:1])
            rt4 = rhs_pair_dyn(ag2_out, 4, "B4s")
            for tt in range(NT):
                sim_iter(n2q, tt, rt4, r22p[:, tt, 1:2])

            # ---------------- final loss ----------------
            r22 = small.tile([P, NT], F32, tag="r22")
            nc.vector.reduce_sum(r22[:], r22p[:], axis=mybir.AxisListType.X)
            c12 = small.tile([P, NT], F32, tag="c12")
            sh11 = small.tile([P, NT], F32, tag="sh11")
            sh22 = small.tile([P, NT], F32, tag="sh22")
            nc.sync.dma_start(c12[:], pt(rs_out[0, :]))
            nc.sync.dma_start(sh11[:], pt(rs_out[1, :]))
            nc.sync.dma_start(sh22[:], pt(rs_out[2, :]))
            p2 = small.tile([P, NT], F32, tag="p2")
            nc.sync.dma_start(p2[:], pt(p_dram[:]))

            d1 = d1p
            d2 = small.tile([P, NT], F32, tag="d2")
            nc.vector.tensor_tensor(d1[:], d1[:], sh11[:], ALU.add)
            nc.vector.tensor_tensor(d2[:], r22[:], sh22[:], ALU.add)
            nc.vector.tensor_tensor(d2[:], d2[:], c12[:], ALU.add)
            nc.vector.tensor_scalar(d2[:], d2[:], -E2, None, ALU.add)
            l1 = small.tile([P, NT], F32, tag="l1")
            l2 = small.tile([P, NT], F32, tag="l2")
            nc.scalar.activation(l1[:], d1[:], AF.Ln)
            nc.scalar.activation(l2[:], d2[:], AF.Ln)
            loss = small.tile([P, NT], F32, tag="loss")
            nc.vector.tensor_tensor(loss[:], l1[:], l2[:], ALU.add)
            nc.vector.tensor_scalar(loss[:], loss[:], 0.5, None, ALU.mult)
            pm = small.tile([P, NT], F32, tag="pm")
            nc.vector.tensor_scalar(pm[:], p2[:], -2.0 * IQS2, None, ALU.mult)
            nc.vector.tensor_tensor(loss[:], loss[:], pm[:], ALU.add)
            nc.sync.dma_start(pt(out[:]), loss[:])

    nc.finalize()
    return nc


@lru_cache(maxsize=1)
def _built():
    return _build()


def _prep_inputs(z1, z2, fc1_w, fc1_b, fc2_w, fc2_b):
    f8 = ml_dtypes.float8_e4m3
    w1tq = np.ascontiguousarray(np.asarray(fc1_w, np.float32).T * WS).astype(f8)
    w2tq = np.ascontiguousarray(np.asarray(fc2_w, np.float32).T * WS).astype(f8)
    b1 = np.asarray(fc1_b, np.float32)
    # fold ELU's -1 through the *quantized* W2 so the fold is exact on device
    b2p = (
        np.asarray(fc2_b, np.float32)
        - w2tq.astype(np.float32).sum(axis=0) * (1.0 / WS)
    ).astype(np.float32)
    in_maps = []
    for c in range(NCORES):
        sl = slice(c * BLK, (c + 1) * BLK)
        in_maps.append(
            {
                "z1t": np.ascontiguousarray(np.asarray(z1[sl], np.float32).T).astype(f8),
                "z2t": np.ascontiguousarray(np.asarray(z2[sl], np.float32).T).astype(f8),
                "w1t": w1tq,
                "w2t": w2tq,
                "b1": b1,
                "b2p": b2p,
            }
        )
    return in_maps


def _install_ntff_shim():
    """Register the axon NTFF profile hook (antenv.axon_hooks is absent in
    this image; rebuild it from trn_agent_boot's ctypes recipe)."""
    import sys
    import types

    if "antenv.axon_hooks" in sys.modules:
        return True
    try:
        import antenv
        from trn_agent_boot.trn_boot import _ntff_profile_via_ctypes

        hook = _ntff_profile_via_ctypes("/opt/axon/libaxon_pjrt.so")
        if hook is None:
            return False
        m = types.ModuleType("antenv.axon_hooks")
        m._hook = hook
        m.get_axon_ntff_profile_hook = lambda: m._hook
        m.set_axon_ntff_profile_hook = lambda h: setattr(m, "_hook", h)
        sys.modules["antenv.axon_hooks"] = m
        antenv.axon_hooks = m
        # artifact upload needs egress; neuter it for local profiling
        import concourse.bass_utils as _bu

        _bu.upload_artifacts = lambda tmpdir: f"file://{tmpdir}"
        return True
    except Exception as e:
        print(f"ntff shim unavailable: {e!r}")
        return False


def _run(in_maps, trace=False):
    nc = _built()
    if trace and not _install_ntff_shim():
        trace = False
    last = None
    for attempt in range(3):
        try:
            res = run_bass_kernel_spmd(nc, in_maps, list(range(NCORES)), trace=trace)
            if all(np.isfinite(res.results[c]["out"]).all() for c in range(NCORES)):
                return res
            print("nonfinite output, retrying")
        except Exception as e:  # device occasionally wedged from a prior process
            last = e
            if "UNRECOVERABLE" not in str(e) and "UNAVAILABLE" not in str(e):
                raise
            print(f"device error (attempt {attempt}): retrying")
    if last is not None:
        raise last
    return res


def kernel(z1, z2, fc1_w, fc1_b, fc2_w, fc2_b):
    in_maps = _prep_inputs(z1, z2, fc1_w, fc1_b, fc2_w, fc2_b)
    res = _run(in_maps, trace=os.environ.get("KERNEL_TRACE", "") == "1")
    if res.exec_time_ns is not None:
        print(f"HW exec time: {res.exec_time_ns} ns")
    out = np.concatenate([res.results[c]["out"] for c in range(NCORES)])
    return out.astype(np.float32)
